# revision 3
# baseline (speedup 1.0000x reference)
"""LocationSensitiveAttention kernel for 8 Trainium2 NeuronCores.

Strategy (batch-parallel over 8 cores, 32 batch rows each):
  - Fold conv weights into W_align on host: tanh-arg = M_loc (x) im2col + qc bias.
  - Per (b, t-window): f32r matmul -> ACT tanh (bias-fused query projection),
    bf16 M=1 score matmuls packed 4-per-PSUM-bank via col-groups,
    ACT exp/sigmoid straight from PSUM, strided-partition DMA gathers.
  - Flash-style unnormalized context accumulation in bf16 against the streamed
    encoded_tokens (bf16), normalized by 1/Z at the end.
"""
import sys
sys.path.insert(0, "/opt/trn_rl_repo")
import numpy as np
import ml_dtypes

import concourse.bass as bass
import concourse.bacc as bacc
import concourse.tile as tile
from concourse import mybir
from concourse.bass_utils import run_bass_kernel_spmd

T, B, H, QH, NF, KS, PAD = 2048, 256, 256, 512, 32, 31, 15
NCORES = 8
BL = B // NCORES          # 32 batch rows per core
TW = 4                    # t-windows
TWS = T // TW             # 512
NCH = T // 128            # 16 context chunks of 128 t

f32 = mybir.dt.float32
f32r = mybir.dt.float32r
bf16 = mybir.dt.bfloat16
bfnp = ml_dtypes.bfloat16

_PROGRAM = None  # (nc, ) built once


def _build_program():
    nc = bacc.Bacc()

    # ---- per-core DRAM inputs ----
    enc_d = nc.dram_tensor("enc", [T, BL, H], bf16, kind="ExternalInput")
    x1_d = nc.dram_tensor("x1", [BL, TW, KS, TWS], f32r, kind="ExternalInput")
    x2_d = nc.dram_tensor("x2", [BL, TW, KS, TWS], bf16, kind="ExternalInput")
    cum_d = nc.dram_tensor("cum", [BL, T], f32, kind="ExternalInput")
    mconv_d = nc.dram_tensor("mconv", [KS, 2, 128], f32r, kind="ExternalInput")
    qT_d = nc.dram_tensor("qT", [128, 4, BL], f32r, kind="ExternalInput")
    wq1_d = nc.dram_tensor("wq1", [128, 4, 2, 128], f32r, kind="ExternalInput")
    bq1_d = nc.dram_tensor("bq1", [128, 2], f32, kind="ExternalInput")
    wq2_d = nc.dram_tensor("wq2", [128, 2, 2, 128], f32r, kind="ExternalInput")
    bias2_d = nc.dram_tensor("bias2", [128, 2], f32, kind="ExternalInput")
    wsc_d = nc.dram_tensor("wsc", [128, 2], bf16, kind="ExternalInput")
    gw_d = nc.dram_tensor("gw", [KS, 1], bf16, kind="ExternalInput")
    gb_d = nc.dram_tensor("gb", [128, 1], f32, kind="ExternalInput")
    ident_d = nc.dram_tensor("ident", [32, 32], f32, kind="ExternalInput")

    ctx_o = nc.dram_tensor("ctx_o", [BL, H], f32, kind="ExternalOutput")
    align_o = nc.dram_tensor("align_o", [BL, T], f32, kind="ExternalOutput")
    cumul_o = nc.dram_tensor("cumul_o", [BL, T], f32, kind="ExternalOutput")

    with tile.TileContext(nc) as tc:
        with tc.tile_pool(name="consts", bufs=1) as consts, \
             tc.tile_pool(name="persist", bufs=1) as persist, \
             tc.tile_pool(name="xp", bufs=3) as xp, \
             tc.tile_pool(name="x2p", bufs=3) as x2p, \
             tc.tile_pool(name="thp", bufs=4) as thp, \
             tc.tile_pool(name="expp", bufs=3) as expp, \
             tc.tile_pool(name="encp", bufs=4) as encp, \
             tc.tile_pool(name="smallp", bufs=2) as smallp, \
             tc.tile_pool(name="ap_", bufs=4) as apool, \
             tc.tile_pool(name="pre_ps", bufs=2, space="PSUM") as pre_ps, \
             tc.tile_pool(name="sg_ps", bufs=3, space="PSUM") as sg_ps, \
             tc.tile_pool(name="ctx_ps", bufs=2, space="PSUM") as ctx_ps:

            # ---- load constants ----
            mconv_sb = consts.tile([KS, 2, 128], f32r)
            qT_sb = consts.tile([128, 4, BL], f32r)
            wq1_sb = consts.tile([128, 4, 2, 128], f32r)
            bq1_sb = consts.tile([128, 2], f32)
            wq2_sb = consts.tile([128, 2, 2, 128], f32r)
            bias2_sb = consts.tile([128, 2], f32)
            wsc_sb = consts.tile([128, 2], bf16)
            gw_sb = consts.tile([KS, 1], bf16)
            gb_sb = consts.tile([128, 1], f32)
            ident_sb = consts.tile([32, 32], f32)
            cum_sb = persist.tile([BL, T], f32, tag="cum")
            for t_, d_ in [(mconv_sb, mconv_d), (qT_sb, qT_d), (wq1_sb, wq1_d),
                           (bq1_sb, bq1_d), (wq2_sb, wq2_d), (bias2_sb, bias2_d),
                           (wsc_sb, wsc_d), (gw_sb, gw_d), (ident_sb, ident_d),
                           (cum_sb, cum_d), (gb_sb, gb_d)]:
                nc.sync.dma_start(out=t_, in_=d_[:])

            # persistent buffers
            exp_tb = persist.tile([BL, T], f32, tag="exp")
            sig_tb = persist.tile([BL, T], f32, tag="sig")
            ctx_parts = persist.tile([BL, TW * H], f32, tag="ctxp")

            # ---- query projection:  qc = Wq2 @ relu(Wq1 @ q + bq1) + bq2 + cb ----
            q1T_sb = []
            qc_sb = []
            for hb in range(2):
                q1t = consts.tile([128, BL], f32r, tag=f"q1_{hb}")
                q1T_sb.append(q1t)
                qct = consts.tile([128, BL], f32, tag=f"qc_{hb}")
                qc_sb.append(qct)
            for hb in range(2):
                psq = pre_ps.tile([128, BL], f32, tag="pre")
                for k in range(4):
                    nc.tensor.matmul(psq, wq1_sb[:, k, hb, :], qT_sb[:, k, :],
                                     start=(k == 0), stop=(k == 3))
                nc.scalar.activation(out=q1T_sb[hb], in_=psq,
                                     func=mybir.ActivationFunctionType.Relu,
                                     bias=bq1_sb[:, hb:hb + 1])
            for gb_i in range(2):
                psq = pre_ps.tile([128, BL], f32, tag="pre")
                for k2 in range(2):
                    nc.tensor.matmul(psq, wq2_sb[:, k2, gb_i, :], q1T_sb[k2],
                                     start=(k2 == 0), stop=(k2 == 1))
                nc.scalar.activation(out=qc_sb[gb_i], in_=psq,
                                     func=mybir.ActivationFunctionType.Identity,
                                     bias=bias2_sb[:, gb_i:gb_i + 1])

            # ---- enc tiles, loaded lazily in chunk order ----
            enc_tiles = {}

            def get_enc(c):
                if c not in enc_tiles:
                    et = encp.tile([128, BL, H], bf16, tag="enc")
                    nc.sync.dma_start(out=et, in_=enc_d[128 * c:128 * (c + 1)])
                    enc_tiles[c] = et
                return enc_tiles[c]

            # ---- main loop ----
            for tw in range(TW):
                for g in range(BL // 4):        # groups of 4 batch rows
                    s4 = sg_ps.tile([128, TWS], f32, tag="sg")
                    g4 = sg_ps.tile([128, TWS], f32, tag="sg")
                    for j in range(4):
                        b = 4 * g + j
                        x1 = xp.tile([KS, TWS], f32r, tag="x1")
                        nc.sync.dma_start(out=x1, in_=x1_d[b, tw])
                        x2 = x2p.tile([KS, TWS], bf16, tag="x2")
                        nc.sync.dma_start(out=x2, in_=x2_d[b, tw])
                        ths = []
                        for hb in range(2):
                            pre = pre_ps.tile([128, TWS], f32, tag="pre")
                            nc.tensor.matmul(pre, mconv_sb[:, hb, :], x1,
                                             start=True, stop=True)
                            th = thp.tile([128, TWS], bf16, tag="th")
                            nc.scalar.activation(
                                out=th, in_=pre,
                                func=mybir.ActivationFunctionType.Tanh,
                                bias=qc_sb[hb][:, b:b + 1])
                            ths.append(th)
                        nc.tensor.matmul(s4[32 * j:32 * j + 1, :],
                                         wsc_sb[:, 0:1], ths[0],
                                         start=True, stop=False,
                                         tile_position=(0, 32 * j))
                        nc.tensor.matmul(s4[32 * j:32 * j + 1, :],
                                         wsc_sb[:, 1:2], ths[1],
                                         start=False, stop=True,
                                         tile_position=(0, 32 * j))
                        nc.tensor.matmul(g4[32 * j:32 * j + 1, :], gw_sb, x2,
                                         start=True, stop=True,
                                         tile_position=(0, 32 * j))
                    exp4 = expp.tile([128, TWS], f32, tag="e4")
                    nc.scalar.activation(out=exp4, in_=s4,
                                         func=mybir.ActivationFunctionType.Exp)
                    sig4 = expp.tile([128, TWS], f32, tag="e4")
                    nc.scalar.activation(out=sig4, in_=g4,
                                         func=mybir.ActivationFunctionType.Sigmoid,
                                         bias=gb_sb)
                    for src, dst in [(exp4, exp_tb), (sig4, sig_tb)]:
                        strided = bass.AP(
                            tensor=src.tensor, offset=src.offset,
                            ap=[[32 * src.ap[0][0], 4]] + [list(x) for x in src.ap[1:]])
                        nc.sync.dma_start(
                            out=dst[4 * g:4 * g + 4, TWS * tw:TWS * (tw + 1)],
                            in_=strided)

                # transpose exp -> a weights [128 t, 32 b] x 4 chunks, as bf16
                trp = ctx_ps.tile([128, 128], f32, tag="ctx")
                for i in range(4):
                    nc.tensor.transpose(
                        trp[:, 32 * i:32 * (i + 1)],
                        exp_tb[:, TWS * tw + 128 * i:TWS * tw + 128 * (i + 1)],
                        ident_sb)
                a_sb = apool.tile([128, 128], bf16, tag="a")
                nc.vector.tensor_copy(a_sb, trp)

                # context accumulation for this window: 8 b per psum bank-tile
                for g8 in range(4):
                    cps = ctx_ps.tile([128, 512], f32, tag="ctx")
                    for j8 in range(8):
                        b = 8 * g8 + j8
                        part = 32 * (j8 % 4)
                        cols = 256 * (j8 // 4)
                        for i in range(4):
                            et = get_enc(4 * tw + i)
                            nc.tensor.matmul(
                                cps[part:part + 1, cols:cols + 256],
                                a_sb[:, 32 * i + b:32 * i + b + 1],
                                et[:, b, :],
                                start=(i == 0), stop=(i == 3),
                                tile_position=(0, part))
                    ctmp = smallp.tile([128, 512], f32, tag="ctmp")
                    nc.vector.tensor_copy(ctmp, cps)
                    for h2 in range(2):
                        strided = bass.AP(
                            tensor=ctmp.tensor, offset=ctmp.offset + 256 * h2,
                            ap=[[32 * ctmp.ap[0][0], 4], [ctmp.ap[1][0], 256]])
                        nc.sync.dma_start(
                            out=ctx_parts[8 * g8 + 4 * h2:8 * g8 + 4 * h2 + 4,
                                          H * tw:H * (tw + 1)],
                            in_=strided)

            # ---- finale ----
            z = persist.tile([BL, 1], f32, tag="z")
            nc.vector.reduce_sum(z, exp_tb, axis=mybir.AxisListType.X)
            rz = persist.tile([BL, 1], f32, tag="rz")
            nc.vector.reciprocal(rz, z)

            align_sb = persist.tile([BL, T], f32, tag="align")
            nc.vector.tensor_scalar_mul(align_sb, exp_tb, rz)
            nc.sync.dma_start(out=align_o[:], in_=align_sb)

            prod = persist.tile([BL, T], f32, tag="prod")
            nc.vector.tensor_mul(prod, align_sb, sig_tb)
            cumul_sb = persist.tile([BL, T], f32, tag="cumul")
            nc.vector.tensor_add(cumul_sb, prod, cum_sb)
            nc.sync.dma_start(out=cumul_o[:], in_=cumul_sb)

            cparts = ctx_parts.rearrange("b (tw h) -> b tw h", tw=TW)
            cs1 = persist.tile([BL, H], f32, tag="cs1")
            nc.vector.tensor_add(cs1, cparts[:, 0, :], cparts[:, 1, :])
            cs2 = persist.tile([BL, H], f32, tag="cs2")
            nc.vector.tensor_add(cs2, cparts[:, 2, :], cparts[:, 3, :])
            cs = persist.tile([BL, H], f32, tag="cs")
            nc.vector.tensor_add(cs, cs1, cs2)
            ctx_sb = persist.tile([BL, H], f32, tag="ctxo")
            nc.vector.tensor_scalar_mul(ctx_sb, cs, rz)
            nc.sync.dma_start(out=ctx_o[:], in_=ctx_sb)

    nc.compile()
    return nc


def _host_prep(encoded_tokens, tokens_mask, query, cumulative_alignment,
               initial_cumulative_alignment, conv_w, conv_b, Wq1, bq1, Wq2, bq2,
               W_align, W_score):
    enc = np.ascontiguousarray(np.asarray(encoded_tokens, np.float32))
    mask = np.asarray(tokens_mask)
    q = np.asarray(query, np.float32)
    cum = np.asarray(cumulative_alignment, np.float32)
    init = np.asarray(initial_cumulative_alignment, np.float32)
    conv_w = np.asarray(conv_w, np.float32)
    conv_b = np.asarray(conv_b, np.float32)
    Wq1 = np.asarray(Wq1, np.float32)
    bq1 = np.asarray(bq1, np.float32)
    Wq2 = np.asarray(Wq2, np.float32)
    bq2 = np.asarray(bq2, np.float32)
    W_align = np.asarray(W_align, np.float32)
    W_score = np.asarray(W_score, np.float32)

    cum_m = np.where(mask, cum, 0.0).astype(np.float32)          # [B, T]
    xpad = np.concatenate([np.repeat(init, PAD, axis=1), cum_m,
                           np.zeros((B, PAD), np.float32)], axis=1)  # [B, T+30]
    sw = np.lib.stride_tricks.sliding_window_view(xpad, KS, axis=1)  # [B, T, KS]
    im2col = np.ascontiguousarray(
        sw.reshape(B, TW, TWS, KS).transpose(0, 1, 3, 2))        # [B, TW, KS, TWS]

    M_loc = W_align @ conv_w[:NF, 0, :]                          # [H, KS]
    cb = W_align @ conv_b[:NF]                                   # [H]
    mconv = np.ascontiguousarray(M_loc.T.reshape(KS, 2, 128))    # [KS, 2, 128]

    qT = np.ascontiguousarray(q[0].T)                            # [QH, B]
    wq1 = np.ascontiguousarray(
        Wq1.T.reshape(4, 128, 2, 128).transpose(1, 0, 2, 3))     # [128, 4, 2, 128]
    bq1_a = np.ascontiguousarray(bq1.reshape(2, 128).T)          # [128, 2]
    wq2 = np.ascontiguousarray(
        Wq2.T.reshape(2, 128, 2, 128).transpose(1, 0, 2, 3))     # [128, 2, 2, 128]
    bias2 = np.ascontiguousarray((bq2 + cb).reshape(2, 128).T)   # [128, 2]
    wsc = np.ascontiguousarray(W_score[0].reshape(2, 128).T).astype(bfnp)
    gw = np.ascontiguousarray(conv_w[NF, 0, :].reshape(KS, 1)).astype(bfnp)
    gb = np.full((128, 1), conv_b[NF], np.float32)
    ident = np.eye(32, dtype=np.float32)
    enc16 = enc.astype(bfnp)                                     # [T, B, H]

    in_maps = []
    for c in range(NCORES):
        bs = slice(BL * c, BL * (c + 1))
        in_maps.append({
            "enc": np.ascontiguousarray(enc16[:, bs, :]),
            "x1": np.ascontiguousarray(im2col[bs]),
            "x2": np.ascontiguousarray(im2col[bs]).astype(bfnp),
            "cum": np.ascontiguousarray(cum_m[bs]),
            "mconv": mconv,
            "qT": np.ascontiguousarray(
                qT[:, bs].reshape(4, 128, BL).transpose(1, 0, 2)),
            "wq1": wq1,
            "bq1": bq1_a,
            "wq2": wq2,
            "bias2": bias2,
            "wsc": wsc,
            "gw": gw,
            "gb": gb,
            "ident": ident,
        })
    return in_maps


def _get_program():
    global _PROGRAM
    if _PROGRAM is None:
        _PROGRAM = _build_program()
    return _PROGRAM


def run(trace=False, **inputs):
    nc = _get_program()
    in_maps = _host_prep(**inputs)
    res = run_bass_kernel_spmd(nc, in_maps, core_ids=list(range(NCORES)),
                               trace=trace)
    ctx = np.concatenate([res.results[c]["ctx_o"] for c in range(NCORES)], axis=0)
    cumul = np.concatenate([res.results[c]["cumul_o"] for c in range(NCORES)], axis=0)
    align = np.concatenate([res.results[c]["align_o"] for c in range(NCORES)], axis=0)
    out = (ctx.astype(np.float32), cumul.astype(np.float32), align.astype(np.float32))
    return out, res


def kernel(**inputs):
    out, _ = run(trace=False, **inputs)
    return out


# revision 4
# speedup vs baseline: 1.0842x; 1.0842x over previous
"""LocationSensitiveAttention kernel for 8 Trainium2 NeuronCores.

Strategy (batch-parallel over 8 cores, 32 batch rows each):
  - Fold conv weights into W_align on host: tanh-arg = M_loc (x) im2col + qc bias.
  - Per (b, t-window): f32r matmul -> ACT tanh (bias-fused query projection),
    bf16 M=1 score matmuls packed 4-per-PSUM-bank via col-groups,
    ACT exp/sigmoid straight from PSUM, strided-partition DMA gathers.
  - Flash-style unnormalized context accumulation in bf16 against the streamed
    encoded_tokens (bf16), normalized by 1/Z at the end.
"""
import sys
sys.path.insert(0, "/opt/trn_rl_repo")
import numpy as np
import ml_dtypes

import concourse.bass as bass
import concourse.bacc as bacc
import concourse.tile as tile
from concourse import mybir
from concourse.bass_utils import run_bass_kernel_spmd

T, B, H, QH, NF, KS, PAD = 2048, 256, 256, 512, 32, 31, 15
NCORES = 8
BL = B // NCORES          # 32 batch rows per core
TW = 4                    # t-windows
TWS = T // TW             # 512
NCH = T // 128            # 16 context chunks of 128 t

f32 = mybir.dt.float32
f32r = mybir.dt.float32r
bf16 = mybir.dt.bfloat16
bfnp = ml_dtypes.bfloat16

_PROGRAM = None  # (nc, ) built once


def _build_program():
    nc = bacc.Bacc()

    # ---- per-core DRAM inputs ----
    enc_d = nc.dram_tensor("enc", [T, BL, H], bf16, kind="ExternalInput")
    x1_d = nc.dram_tensor("x1", [TW, BL // 4, KS, 4 * TWS], f32r, kind="ExternalInput")
    x2_d = nc.dram_tensor("x2", [TW, BL // 4, KS, 4 * TWS], bf16, kind="ExternalInput")
    cum_d = nc.dram_tensor("cum", [BL, T], f32, kind="ExternalInput")
    mconv_d = nc.dram_tensor("mconv", [KS, 2, 128], f32r, kind="ExternalInput")
    qT_d = nc.dram_tensor("qT", [128, 4, BL], f32r, kind="ExternalInput")
    wq1_d = nc.dram_tensor("wq1", [128, 4, 2, 128], f32r, kind="ExternalInput")
    bq1_d = nc.dram_tensor("bq1", [128, 2], f32, kind="ExternalInput")
    wq2_d = nc.dram_tensor("wq2", [128, 2, 2, 128], f32r, kind="ExternalInput")
    bias2_d = nc.dram_tensor("bias2", [128, 2], f32, kind="ExternalInput")
    wsc_d = nc.dram_tensor("wsc", [128, 2], bf16, kind="ExternalInput")
    gw_d = nc.dram_tensor("gw", [KS, 1], bf16, kind="ExternalInput")
    gb_d = nc.dram_tensor("gb", [128, 1], f32, kind="ExternalInput")
    ident_d = nc.dram_tensor("ident", [32, 32], f32, kind="ExternalInput")

    ctx_o = nc.dram_tensor("ctx_o", [BL, H], f32, kind="ExternalOutput")
    align_o = nc.dram_tensor("align_o", [BL, T], f32, kind="ExternalOutput")
    cumul_o = nc.dram_tensor("cumul_o", [BL, T], f32, kind="ExternalOutput")

    with tile.TileContext(nc) as tc:
        with tc.tile_pool(name="consts", bufs=1) as consts, \
             tc.tile_pool(name="persist", bufs=1) as persist, \
             tc.tile_pool(name="xp", bufs=3) as xp, \
             tc.tile_pool(name="x2p", bufs=3) as x2p, \
             tc.tile_pool(name="thp", bufs=4) as thp, \
             tc.tile_pool(name="expp", bufs=3) as expp, \
             tc.tile_pool(name="encp", bufs=4) as encp, \
             tc.tile_pool(name="smallp", bufs=2) as smallp, \
             tc.tile_pool(name="ap_", bufs=4) as apool, \
             tc.tile_pool(name="pre_ps", bufs=2, space="PSUM") as pre_ps, \
             tc.tile_pool(name="sg_ps", bufs=3, space="PSUM") as sg_ps, \
             tc.tile_pool(name="ctx_ps", bufs=2, space="PSUM") as ctx_ps:

            # ---- load constants ----
            mconv_sb = consts.tile([KS, 2, 128], f32r)
            qT_sb = consts.tile([128, 4, BL], f32r)
            wq1_sb = consts.tile([128, 4, 2, 128], f32r)
            bq1_sb = consts.tile([128, 2], f32)
            wq2_sb = consts.tile([128, 2, 2, 128], f32r)
            bias2_sb = consts.tile([128, 2], f32)
            wsc_sb = consts.tile([128, 2], bf16)
            gw_sb = consts.tile([KS, 1], bf16)
            gb_sb = consts.tile([128, 1], f32)
            ident_sb = consts.tile([32, 32], f32)
            cum_sb = persist.tile([BL, T], f32, tag="cum")
            for t_, d_ in [(mconv_sb, mconv_d), (qT_sb, qT_d), (wq1_sb, wq1_d),
                           (bq1_sb, bq1_d), (wq2_sb, wq2_d), (bias2_sb, bias2_d),
                           (wsc_sb, wsc_d), (gw_sb, gw_d), (ident_sb, ident_d),
                           (cum_sb, cum_d), (gb_sb, gb_d)]:
                nc.sync.dma_start(out=t_, in_=d_[:])

            # persistent buffers
            exp_tb = persist.tile([BL, T], f32, tag="exp")
            s_tb = persist.tile([BL, T], f32, tag="sraw")
            g_tb = persist.tile([BL, T], f32, tag="graw")
            sig_tb = persist.tile([BL, T], f32, tag="sig")
            ctx_parts = persist.tile([BL, TW * H], f32, tag="ctxp")

            # ---- query projection:  qc = Wq2 @ relu(Wq1 @ q + bq1) + bq2 + cb ----
            q1T_sb = []
            qc_sb = []
            for hb in range(2):
                q1t = consts.tile([128, BL], f32r, tag=f"q1_{hb}")
                q1T_sb.append(q1t)
                qct = consts.tile([128, BL], f32, tag=f"qc_{hb}")
                qc_sb.append(qct)
            for hb in range(2):
                psq = pre_ps.tile([128, BL], f32, tag="pre")
                for k in range(4):
                    nc.tensor.matmul(psq, wq1_sb[:, k, hb, :], qT_sb[:, k, :],
                                     start=(k == 0), stop=(k == 3))
                nc.scalar.activation(out=q1T_sb[hb], in_=psq,
                                     func=mybir.ActivationFunctionType.Relu,
                                     bias=bq1_sb[:, hb:hb + 1])
            for gb_i in range(2):
                psq = pre_ps.tile([128, BL], f32, tag="pre")
                for k2 in range(2):
                    nc.tensor.matmul(psq, wq2_sb[:, k2, gb_i, :], q1T_sb[k2],
                                     start=(k2 == 0), stop=(k2 == 1))
                nc.scalar.activation(out=qc_sb[gb_i], in_=psq,
                                     func=mybir.ActivationFunctionType.Identity,
                                     bias=bias2_sb[:, gb_i:gb_i + 1])

            # ---- enc tiles, loaded lazily in chunk order ----
            enc_tiles = {}

            def get_enc(c):
                if c not in enc_tiles:
                    et = encp.tile([128, BL, H], bf16, tag="enc")
                    nc.scalar.dma_start(out=et, in_=enc_d[128 * c:128 * (c + 1)])
                    enc_tiles[c] = et
                return enc_tiles[c]

            # ---- main loop ----
            for tw in range(TW):
                for g in range(BL // 4):        # groups of 4 batch rows
                    s4 = sg_ps.tile([128, TWS], f32, tag="sg")
                    g4 = sg_ps.tile([128, TWS], f32, tag="sg")
                    x1 = xp.tile([KS, 4 * TWS], f32r, tag="x1")
                    nc.sync.dma_start(out=x1, in_=x1_d[tw, g])
                    x2 = x2p.tile([KS, 4 * TWS], bf16, tag="x2")
                    nc.scalar.dma_start(out=x2, in_=x2_d[tw, g])
                    for j in range(4):
                        b = 4 * g + j
                        xs = slice(TWS * j, TWS * (j + 1))
                        ths = []
                        for hb in range(2):
                            pre = pre_ps.tile([128, TWS], f32, tag="pre")
                            nc.tensor.matmul(pre, mconv_sb[:, hb, :], x1[:, xs],
                                             start=True, stop=True)
                            th = thp.tile([128, TWS], bf16, tag="th")
                            nc.scalar.activation(
                                out=th, in_=pre,
                                func=mybir.ActivationFunctionType.Tanh,
                                bias=qc_sb[hb][:, b:b + 1])
                            ths.append(th)
                        nc.tensor.matmul(s4[32 * j:32 * j + 1, :],
                                         wsc_sb[:, 0:1], ths[0],
                                         start=True, stop=False,
                                         tile_position=(0, 32 * j))
                        nc.tensor.matmul(s4[32 * j:32 * j + 1, :],
                                         wsc_sb[:, 1:2], ths[1],
                                         start=False, stop=True,
                                         tile_position=(0, 32 * j))
                        nc.tensor.matmul(g4[32 * j:32 * j + 1, :], gw_sb,
                                         x2[:, xs],
                                         start=True, stop=True,
                                         tile_position=(0, 32 * j))
                    s4c = expp.tile([128, TWS], f32, tag="e4")
                    nc.vector.tensor_copy(s4c, s4)
                    g4c = expp.tile([128, TWS], f32, tag="e4")
                    nc.vector.tensor_copy(g4c, g4)
                    for srt, dst in [(s4c, s_tb), (g4c, g_tb)]:
                        strided = bass.AP(
                            tensor=srt.tensor, offset=srt.offset,
                            ap=[[32 * srt.ap[0][0], 4]] + [list(x) for x in srt.ap[1:]])
                        nc.sync.dma_start(
                            out=dst[4 * g:4 * g + 4, TWS * tw:TWS * (tw + 1)],
                            in_=strided)
                # exp for this window (b-major), feeds transposes + finale
                nc.scalar.activation(
                    out=exp_tb[:, TWS * tw:TWS * (tw + 1)],
                    in_=s_tb[:, TWS * tw:TWS * (tw + 1)],
                    func=mybir.ActivationFunctionType.Exp)

                # transpose exp -> a weights [128 t, 32 b] x 4 chunks, as bf16
                trp = ctx_ps.tile([128, 128], f32, tag="ctx")
                for i in range(4):
                    nc.tensor.transpose(
                        trp[:, 32 * i:32 * (i + 1)],
                        exp_tb[:, TWS * tw + 128 * i:TWS * tw + 128 * (i + 1)],
                        ident_sb)
                a_sb = apool.tile([128, 128], bf16, tag="a")
                nc.vector.tensor_copy(a_sb, trp)

                # context accumulation for this window: 8 b per psum bank-tile
                for g8 in range(4):
                    cps = ctx_ps.tile([128, 512], f32, tag="ctx")
                    for j8 in range(8):
                        b = 8 * g8 + j8
                        part = 32 * (j8 % 4)
                        cols = 256 * (j8 // 4)
                        for i in range(4):
                            et = get_enc(4 * tw + i)
                            nc.tensor.matmul(
                                cps[part:part + 1, cols:cols + 256],
                                a_sb[:, 32 * i + b:32 * i + b + 1],
                                et[:, b, :],
                                start=(i == 0), stop=(i == 3),
                                tile_position=(0, part))
                    ctmp = smallp.tile([128, 512], f32, tag="ctmp")
                    nc.vector.tensor_copy(ctmp, cps)
                    for h2 in range(2):
                        strided = bass.AP(
                            tensor=ctmp.tensor, offset=ctmp.offset + 256 * h2,
                            ap=[[32 * ctmp.ap[0][0], 4], [ctmp.ap[1][0], 256]])
                        nc.sync.dma_start(
                            out=ctx_parts[8 * g8 + 4 * h2:8 * g8 + 4 * h2 + 4,
                                          H * tw:H * (tw + 1)],
                            in_=strided)

            # ---- finale ----
            z = persist.tile([BL, 1], f32, tag="z")
            nc.vector.reduce_sum(z, exp_tb, axis=mybir.AxisListType.X)
            rz = persist.tile([BL, 1], f32, tag="rz")
            nc.vector.reciprocal(rz, z)

            align_sb = persist.tile([BL, T], f32, tag="align")
            nc.vector.tensor_scalar_mul(align_sb, exp_tb, rz)
            nc.sync.dma_start(out=align_o[:], in_=align_sb)

            nc.scalar.activation(out=sig_tb, in_=g_tb,
                                 func=mybir.ActivationFunctionType.Sigmoid,
                                 bias=gb_sb[0:BL])
            prod = persist.tile([BL, T], f32, tag="prod")
            nc.vector.tensor_mul(prod, align_sb, sig_tb)
            cumul_sb = persist.tile([BL, T], f32, tag="cumul")
            nc.vector.tensor_add(cumul_sb, prod, cum_sb)
            nc.sync.dma_start(out=cumul_o[:], in_=cumul_sb)

            cparts = ctx_parts.rearrange("b (tw h) -> b tw h", tw=TW)
            cs1 = persist.tile([BL, H], f32, tag="cs1")
            nc.vector.tensor_add(cs1, cparts[:, 0, :], cparts[:, 1, :])
            cs2 = persist.tile([BL, H], f32, tag="cs2")
            nc.vector.tensor_add(cs2, cparts[:, 2, :], cparts[:, 3, :])
            cs = persist.tile([BL, H], f32, tag="cs")
            nc.vector.tensor_add(cs, cs1, cs2)
            ctx_sb = persist.tile([BL, H], f32, tag="ctxo")
            nc.vector.tensor_scalar_mul(ctx_sb, cs, rz)
            nc.sync.dma_start(out=ctx_o[:], in_=ctx_sb)

    nc.compile()
    return nc


def _host_prep(encoded_tokens, tokens_mask, query, cumulative_alignment,
               initial_cumulative_alignment, conv_w, conv_b, Wq1, bq1, Wq2, bq2,
               W_align, W_score):
    enc = np.ascontiguousarray(np.asarray(encoded_tokens, np.float32))
    mask = np.asarray(tokens_mask)
    q = np.asarray(query, np.float32)
    cum = np.asarray(cumulative_alignment, np.float32)
    init = np.asarray(initial_cumulative_alignment, np.float32)
    conv_w = np.asarray(conv_w, np.float32)
    conv_b = np.asarray(conv_b, np.float32)
    Wq1 = np.asarray(Wq1, np.float32)
    bq1 = np.asarray(bq1, np.float32)
    Wq2 = np.asarray(Wq2, np.float32)
    bq2 = np.asarray(bq2, np.float32)
    W_align = np.asarray(W_align, np.float32)
    W_score = np.asarray(W_score, np.float32)

    cum_m = np.where(mask, cum, 0.0).astype(np.float32)          # [B, T]
    xpad = np.concatenate([np.repeat(init, PAD, axis=1), cum_m,
                           np.zeros((B, PAD), np.float32)], axis=1)  # [B, T+30]
    sw = np.lib.stride_tricks.sliding_window_view(xpad, KS, axis=1)  # [B, T, KS]
    # [B, TW, KS, TWS] -> per-core later sliced & grouped to [TW, 8, KS, 4*TWS]
    im2col = sw.reshape(B, TW, TWS, KS).transpose(0, 1, 3, 2)

    M_loc = W_align @ conv_w[:NF, 0, :]                          # [H, KS]
    cb = W_align @ conv_b[:NF]                                   # [H]
    mconv = np.ascontiguousarray(M_loc.T.reshape(KS, 2, 128))    # [KS, 2, 128]

    qT = np.ascontiguousarray(q[0].T)                            # [QH, B]
    wq1 = np.ascontiguousarray(
        Wq1.T.reshape(4, 128, 2, 128).transpose(1, 0, 2, 3))     # [128, 4, 2, 128]
    bq1_a = np.ascontiguousarray(bq1.reshape(2, 128).T)          # [128, 2]
    wq2 = np.ascontiguousarray(
        Wq2.T.reshape(2, 128, 2, 128).transpose(1, 0, 2, 3))     # [128, 2, 2, 128]
    bias2 = np.ascontiguousarray((bq2 + cb).reshape(2, 128).T)   # [128, 2]
    wsc = np.ascontiguousarray(W_score[0].reshape(2, 128).T).astype(bfnp)
    gw = np.ascontiguousarray(conv_w[NF, 0, :].reshape(KS, 1)).astype(bfnp)
    gb = np.full((128, 1), conv_b[NF], np.float32)
    ident = np.eye(32, dtype=np.float32)
    enc16 = enc.astype(bfnp)                                     # [T, B, H]

    in_maps = []
    for c in range(NCORES):
        bs = slice(BL * c, BL * (c + 1))
        # [32, TW, KS, TWS] -> [TW, 8, KS, 4, TWS] -> [TW, 8, KS, 4*TWS]
        x1c = np.ascontiguousarray(
            im2col[bs].reshape(8, 4, TW, KS, TWS).transpose(2, 0, 3, 1, 4)
            .reshape(TW, 8, KS, 4 * TWS))
        in_maps.append({
            "enc": np.ascontiguousarray(enc16[:, bs, :]),
            "x1": x1c,
            "x2": x1c.astype(bfnp),
            "cum": np.ascontiguousarray(cum_m[bs]),
            "mconv": mconv,
            "qT": np.ascontiguousarray(
                qT[:, bs].reshape(4, 128, BL).transpose(1, 0, 2)),
            "wq1": wq1,
            "bq1": bq1_a,
            "wq2": wq2,
            "bias2": bias2,
            "wsc": wsc,
            "gw": gw,
            "gb": gb,
            "ident": ident,
        })
    return in_maps


def _get_program():
    global _PROGRAM
    if _PROGRAM is None:
        _PROGRAM = _build_program()
    return _PROGRAM


def run(trace=False, **inputs):
    nc = _get_program()
    in_maps = _host_prep(**inputs)
    res = run_bass_kernel_spmd(nc, in_maps, core_ids=list(range(NCORES)),
                               trace=trace)
    ctx = np.concatenate([res.results[c]["ctx_o"] for c in range(NCORES)], axis=0)
    cumul = np.concatenate([res.results[c]["cumul_o"] for c in range(NCORES)], axis=0)
    align = np.concatenate([res.results[c]["align_o"] for c in range(NCORES)], axis=0)
    out = (ctx.astype(np.float32), cumul.astype(np.float32), align.astype(np.float32))
    return out, res


def kernel(**inputs):
    out, _ = run(trace=False, **inputs)
    return out
